# revision 16
# baseline (speedup 1.0000x reference)
"""Trainium2 Bass kernel for nn_Affine_Linear_22067541967103.

Math (per point p = (b, n, d), vectors in R^3):
    a1 = J[p,:,0], a2 = J[p,:,1], x = X[p,:]
    The Gram-Schmidt frame (b1,b2,b3) is orthonormal, so the reference
    reduces to (b3 = normalize(a1 x a2)):
        c_term = b3 (b3 . x)
        b_term = b3 x x
        a_term = x - c_term
    Y[b,n] = A @ X + (C-A) @ c_term + Bm @ b_term     (contraction over d)

Device computation per point:
    c  = a1 x a2            (fp32 inputs/products, cancellation-safe)
    s  = |c|^2 (fp32), r = rsqrt(s)/16 (fp16)
    cr = c * r  (= b3/16), H[j,k] = cr_j * x_k, t = sum_k H[k,k]
    ct = cr * t (= c_term/256)
    b_term_i/16 = H[a,b] - H[b,a] folded into the matmul via +16Bm / -16Bm;
    ct scaling folded via 256(C-A).

Layout: core c handles batch b=c. Partition p = h*64 + d (h = n-half),
free axis j: n = h*4096 + j. Inputs are host-packed per chunk as bytes:
6 fp32 J planes then 3 fp16 X planes, [NCH, 128, 30*T] uint8.

Engine notes (measured on this setup): GPSIMD tensor_tensor and any
DVE op with a broadcast (step-0) AP are pathologically slow - avoided.
All elementwise runs on DVE with contiguous APs; ACT does squares,
rsqrt and PSUM evacuation; PE does the 12 accumulating matmuls/chunk.
"""

import numpy as np

B, N, D, F = 8, 8192, 64, 64
NCORES = 8
NHALF = N // 2           # 4096 free columns per core
T = 1024                 # chunk columns
NCH = NHALF // T
MM_FREE = 512            # PSUM bank free-dim limit (fp32)

# J plane order in DRAM: [a1_1, a1_2, a1_0, a2_2, a2_0, a2_1]
# so that pa_k = a1_{k+1} * a2_{k+2} is one fused [128,3,T] multiply.
_J_PLANES = [(1, 0), (2, 0), (0, 0), (2, 1), (0, 1), (1, 1)]  # (k, c)

_CACHE: dict = {}


def _build_nc(repeat=1, skip_mm=False, tiny_io=False, dyn_repeat=None):
    import concourse.bacc as bacc
    import concourse.bass as bass
    import concourse.tile as tile
    import concourse.mybir as mybir
    from contextlib import ExitStack

    dt = mybir.dt
    nc = bacc.Bacc("TRN2", target_bir_lowering=False, debug=False, num_devices=NCORES)

    jx_nch = 1 if tiny_io else NCH
    jx = nc.dram_tensor("jx", [jx_nch, 128, 30 * T], dt.uint8, kind="ExternalInput")
    wt = nc.dram_tensor("wt", [4, 128, 128], dt.float16, kind="ExternalInput")
    yt_cols = T if tiny_io else NHALF
    yt = nc.dram_tensor("yt", [3, 128, yt_cols], dt.float16, kind="ExternalOutput")

    AF = mybir.ActivationFunctionType
    # (a, b) pairs per output component i: bt_i = H[a][b] - H[b][a]
    AB = [(1, 2), (2, 0), (0, 1)]
    # pb_k = a1_{k+2} * a2_{k+1}: slot pairs in jin (see _J_PLANES):
    PB_SLOTS = [(1, 5), (2, 3), (0, 4)]

    with tile.TileContext(nc) as tc, ExitStack() as ctx:
        pool = ctx.enter_context(tc.tile_pool(name="main", bufs=1))
        psum = ctx.enter_context(tc.tile_pool(name="psum", bufs=1, space="PSUM"))

        wtile = pool.tile([128, 4, 128], dt.float16, tag="wt", bufs=1)
        nc.sync.dma_start(wtile[:], wt.ap().rearrange("w p m -> p w m"))

        from contextlib import nullcontext
        loop_cm = tc.For_i(0, dyn_repeat, 1) if dyn_repeat else nullcontext()
        with loop_cm:
         for rep in range(repeat):
          for ch in range(NCH):
            sfx = f"{rep}_{ch}"
            cols = slice(ch * T, (ch + 1) * T)

            inbuf = pool.tile([128, 30 * T], dt.uint8, tag="inbuf", bufs=2,
                              name=f"inbuf{sfx}")
            nc.sync.dma_start(inbuf[:], jx.ap()[0 if tiny_io else ch])
            jin = inbuf[:, 0:24 * T].bitcast(dt.float32).rearrange(
                "p (m t) -> p m t", m=6)
            xin = inbuf[:, 24 * T:30 * T].bitcast(dt.float16).rearrange(
                "p (m t) -> p m t", m=3)

            # cross products in fp32 (DVE)
            pa = pool.tile([128, 3, T], dt.float32, tag="pa", bufs=1, name=f"pa{sfx}")
            nc.vector.tensor_mul(pa[:], jin[:, 0:3, :], jin[:, 3:6, :])
            pb = pool.tile([128, 3, T], dt.float32, tag="pb", bufs=1, name=f"pb{sfx}")
            for k, (s1, s2) in enumerate(PB_SLOTS):
                nc.vector.tensor_mul(pb[:, k, :], jin[:, s1, :], jin[:, s2, :])
            c = pool.tile([128, 3, T], dt.float16, tag="c", bufs=2, name=f"c{sfx}")
            nc.vector.tensor_sub(c[:], pa[:], pb[:])

            # s = |c|^2 in fp32 (squares on ACT)
            sq = pool.tile([128, 3, T], dt.float32, tag="sq", bufs=1, name=f"sq{sfx}")
            nc.scalar.square(sq[:], c[:])
            s01 = pool.tile([128, T], dt.float32, tag="s01", bufs=1, name=f"s01{sfx}")
            nc.vector.tensor_add(s01[:], sq[:, 0, :], sq[:, 1, :])
            s = pool.tile([128, T], dt.float32, tag="s", bufs=1, name=f"s{sfx}")
            nc.vector.tensor_add(s[:], s01[:], sq[:, 2, :])

            # r' = 1/sqrt(s'/16) = rsqrt(s)/16 given the x8 input prescale
            # (s' = 4096 s). All fp16 tensors are scaled to avoid denormals,
            # which stall the DVE; scales are folded into the weights.
            # Abs_reciprocal_sqrt measures ~4e-5 rel err.
            r = pool.tile([128, T], dt.float16, tag="r", bufs=1, name=f"r{sfx}")
            nc.scalar.activation(r[:], s[:], AF.Abs_reciprocal_sqrt, scale=1.0 / 16.0)

            # cr = c * r (per-component: broadcast APs are a DVE cliff)
            cr = pool.tile([128, 3, T], dt.float16, tag="cr", bufs=1, name=f"cr{sfx}")
            for k in range(3):
                nc.vector.tensor_mul(cr[:, k, :], c[:, k, :], r[:])

            # H[j][:, k, :] = cr_j * x_k
            H = []
            for j in range(3):
                Hj = pool.tile([128, 3, T], dt.float16, tag=f"H{j}", bufs=2,
                               name=f"H{j}_{sfx}")
                for k in range(3):
                    nc.vector.tensor_mul(Hj[:, k, :], cr[:, j, :], xin[:, k, :])
                H.append(Hj)

            # t = b3 . x / 16
            t01 = pool.tile([128, T], dt.float16, tag="t01", bufs=1, name=f"t01{sfx}")
            nc.vector.tensor_add(t01[:], H[0][:, 0, :], H[1][:, 1, :])
            t = pool.tile([128, T], dt.float16, tag="t", bufs=1, name=f"t{sfx}")
            nc.vector.tensor_add(t[:], t01[:], H[2][:, 2, :])

            # ct = cr * t
            ct = pool.tile([128, 3, T], dt.float16, tag="ct", bufs=2, name=f"ct{sfx}")
            for k in range(3):
                nc.vector.tensor_mul(ct[:, k, :], cr[:, k, :], t[:])

            yout = pool.tile([128, 3, T], dt.float16, tag="yout", bufs=2,
                             name=f"yout{sfx}")
            if skip_mm:
                nc.vector.tensor_copy(yout[:], ct[:])
            for sl in range(0 if skip_mm else T // MM_FREE):
                scol = slice(sl * MM_FREE, (sl + 1) * MM_FREE)
                ps = []
                for i in range(3):
                    p_i = psum.tile([128, MM_FREE], dt.float32, tag=f"ps{i}",
                                    bufs=2, name=f"ps{i}_{sfx}_{sl}")
                    ps.append(p_i)
                # weight-major issue order to minimize LDWEIGHTS
                for i in range(3):
                    nc.tensor.matmul(ps[i][:], wtile[:, 0, :], xin[:, i, scol],
                                     start=True, stop=False)
                for i in range(3):
                    nc.tensor.matmul(ps[i][:], wtile[:, 1, :], ct[:, i, scol],
                                     start=False, stop=False)
                for i in range(3):
                    a, b = AB[i]
                    nc.tensor.matmul(ps[i][:], wtile[:, 2, :], H[a][:, b, scol],
                                     start=False, stop=False)
                for i in range(3):
                    a, b = AB[i]
                    nc.tensor.matmul(ps[i][:], wtile[:, 3, :], H[b][:, a, scol],
                                     start=False, stop=True)
                for i in range(3):
                    nc.scalar.copy(yout[:, i, scol], ps[i][:])

            ocols = slice(0, T) if tiny_io else cols
            nc.sync.dma_start(yt.ap()[:, :, ocols].rearrange("m p t -> p m t"),
                              yout[:])

    nc.compile()
    return nc


def _plane(arr2d):
    """[8192, 64] -> [128, 4096] with p = h*64+d, j = n%4096."""
    return np.ascontiguousarray(
        arr2d.reshape(2, NHALF, D).transpose(0, 2, 1).reshape(128, NHALF)
    )


def _pack_core(Jb, Xb):
    # x8 prescale keeps every device fp16 tensor away from the denormal
    # range (DVE stalls hard on subnormal inputs); compensated in weights.
    jt = np.empty((128, 6, NHALF), dtype=np.float32)
    for m, (k, cc) in enumerate(_J_PLANES):
        jt[:, m, :] = _plane(8.0 * Jb[:, :, k, cc])
    xt = np.empty((128, 3, NHALF), dtype=np.float16)
    for i in range(3):
        xt[:, i, :] = _plane(8.0 * Xb[:, :, i]).astype(np.float16)
    # pack into [NCH, 128, 30T] bytes: per chunk, 6*T fp32 then 3*T fp16
    jx = np.empty((NCH, 128, 30 * T), dtype=np.uint8)
    for ch in range(NCH):
        cols = slice(ch * T, (ch + 1) * T)
        jb = np.ascontiguousarray(jt[:, :, cols]).reshape(128, 6 * T).view(np.uint8)
        xb = np.ascontiguousarray(xt[:, :, cols]).reshape(128, 3 * T).view(np.uint8)
        jx[ch, :, :24 * T] = jb
        jx[ch, :, 24 * T:] = xb
    return jx


def _blockdiag_T(W):
    out = np.zeros((128, 128), dtype=np.float16)
    out[:64, :64] = W.T.astype(np.float16)
    out[64:, 64:] = W.T.astype(np.float16)
    return out


def kernel(X, J, A, Bm, C):
    if "nc" not in _CACHE:
        _CACHE["nc"] = _build_nc()
    nc = _CACHE["nc"]

    X = np.asarray(X)
    J = np.asarray(J)
    # device values: x' = 8x, cr = 4 b3, t' = 32 (b3.x), ct' = 128 c_term,
    # H cross terms = 32 b_term - compensate in the weights.
    wts = np.stack([
        _blockdiag_T(np.asarray(A) / 8.0),
        _blockdiag_T((np.asarray(C) - np.asarray(A)) / 128.0),
        _blockdiag_T(np.asarray(Bm) / 32.0),
        _blockdiag_T(-np.asarray(Bm) / 32.0),
    ])

    in_maps = []
    for b in range(NCORES):
        jxb = _pack_core(J[b], X[b])
        in_maps.append({"jx": jxb, "wt": wts})

    from concourse import bass_utils
    res = bass_utils.run_bass_kernel_spmd(nc, in_maps, core_ids=list(range(NCORES)))

    Y = np.empty((B, N, F, 3), dtype=np.float32)
    for b in range(NCORES):
        ytb = res.results[b]["yt"].astype(np.float32)  # [3, 128, 4096]
        Y[b] = (ytb.reshape(3, 2, F, NHALF)
                   .transpose(1, 3, 2, 0)
                   .reshape(N, F, 3))
    return Y
